# revision 15
# baseline (speedup 1.0000x reference)
"""MeshConv (gnn_message_passing) Trainium2 kernel.

Math (per batch b):
    idx[e] = [e, ne0[e], ne1[e], ne2[e], ne3[e]]   (self + 4 neighbor edges)
    taps:  e0 = x[:, e],  ek = x[:, ne_{k-1}[e]]
    G = [e0, e1+e3, e2+e4, |e1-e3|, |e2-e4|]       (5 "taps" of 128 channels)
    out[o, e] = sum_{c,k} G[c, e, k] * W[o, c, k] + bias[o]

Strategy (8 NeuronCores):
  - Data parallel over (batch, edge-half): core = b*2 + h handles 15000 edges
    of batch b. Conv weight replicated.
  - Neighbor taps fetched with SWDGE dma_gather(transpose=True) from a
    host-pretransposed [E, 128] bf16 copy of x: each gather lands a
    [128 channels, N edges] bf16 tile directly in matmul-rhs layout
    (no on-chip transposes needed).
  - Tap combines (add / sub) on DVE in bf16, |.| on ACT.
  - 5 accumulating bf16 matmuls per 128-output-half into fp32 PSUM; bias is
    fused into the PSUM->SBUF copy on ACT (Identity activation with bias AP).
"""

import os
import sys

sys.path.insert(0, "/opt/trn_rl_repo")

from contextlib import ExitStack

import ml_dtypes
import numpy as np

import concourse.bacc as bacc
import concourse.bass as bass
import concourse.tile as tile
from concourse import mybir

BF16 = ml_dtypes.bfloat16

P = 128          # partitions / in-channels
B, C, E, KT = 4, 128, 30000, 5
CO = 256         # out-channels
NCORES = 8
EH = E // 2      # edges per core (half a batch)
TILE = 3840      # max edges per gather macro-tile (multiple of 128)
# Measured SWDGE gather cost: ~0.70us fixed + ~7.8-9.5ns/idx (state-
# dependent). Big tiles minimize the per-instruction fixed cost; the last
# tile is split in two so the post-last-gather pipeline (combine + matmul
# + evict of the final tile = the kernel tail) is halved.
# sum = 15104 >= EH (minimal 128-multiple).
TSZ = (3840, 3840, 3840, 2816, 768)
NT = 5
EPAD = sum(TSZ)
CH = 480         # matmul chunk: psum free dim (480 f32 <= one 2KB bank)
IDXW = TILE // 16  # idx wrap columns per tap (padded; tile 3 uses 224)

_LAST_RESULTS = None  # BassKernelResults of the most recent run (for test.py)
_PROGRAM = None


def build_program(nt: int = NT) -> bass.Bass:
    # NOTE: num_swdge_queues > 1 with queue_num round-robin measured 1.3x
    # faster gather generation but produces corrupted gather data on HW
    # (completion semaphores misfire across queues) — keep a single queue.
    nc = bacc.Bacc("TRN2")
    xt = nc.declare_dram_parameter("xt", [E, C], mybir.dt.bfloat16, isOutput=False)
    x0 = nc.declare_dram_parameter("x0", [C, EPAD], mybir.dt.bfloat16, isOutput=False)
    idx = nc.declare_dram_parameter(
        "idx", [P, NT * 4 * IDXW], mybir.dt.int16, isOutput=False
    )
    wt = nc.declare_dram_parameter("wt", [P, KT * CO], mybir.dt.bfloat16, isOutput=False)
    bias = nc.declare_dram_parameter("bias", [P, 2], mybir.dt.float32, isOutput=False)
    out = nc.declare_dram_parameter("out", [CO, EH], mybir.dt.float32, isOutput=True)

    with tile.TileContext(nc) as tc, ExitStack() as ctx:
        consts = ctx.enter_context(tc.tile_pool(name="consts", bufs=1))
        gpool = ctx.enter_context(tc.tile_pool(name="gath", bufs=2))
        cpool = ctx.enter_context(tc.tile_pool(name="comb", bufs=2))
        opool = ctx.enter_context(tc.tile_pool(name="outs", bufs=4))
        psum = ctx.enter_context(tc.tile_pool(name="psum", bufs=3, space="PSUM"))

        # Idx prefetch split in two: tile 0's segment lands first so the
        # first gather (the head of the GpSimd critical path) is not gated
        # on the full index transfer; the rest follows in one bulk DMA.
        idx0_t = consts.tile([P, 4 * IDXW], mybir.dt.int16, tag="idx0")
        nc.sync.dma_start(out=idx0_t[:], in_=idx[:, : 4 * IDXW])
        idxr_t = consts.tile([P, (nt - 1) * 4 * IDXW], mybir.dt.int16, tag="idxr")
        nc.sync.dma_start(out=idxr_t[:], in_=idx[:, 4 * IDXW : nt * 4 * IDXW])
        wt_t = consts.tile([P, KT * CO], mybir.dt.bfloat16)
        nc.scalar.dma_start(out=wt_t[:], in_=wt[:])
        bias_t = consts.tile([P, 2], mybir.dt.float32)
        nc.scalar.dma_start(out=bias_t[:], in_=bias[:])

        toff = [sum(TSZ[:i]) for i in range(nt + 1)]
        for t in range(nt):
            sz = TSZ[t]
            szw = sz // 16
            idx_t = idx0_t if t == 0 else idxr_t
            ib = 0 if t == 0 else (t - 1) * 4 * IDXW
            x0_t = gpool.tile([P, TILE], mybir.dt.bfloat16, tag="x0")
            nc.scalar.dma_start(
                out=x0_t[:, :sz], in_=x0[:, toff[t] : toff[t] + sz]
            )

            g = [None] * 4
            # tap order (0,2,1,3): each DVE combine's operand pair completes
            # one gather sooner, so combines overlap the remaining gathers.
            for k in (0, 2, 1, 3):
                gk = gpool.tile([P, TILE], mybir.dt.bfloat16, tag=f"g{k}")
                nc.gpsimd.dma_gather(
                    gk[:, :sz].rearrange("p (a n) -> p a n", a=1),
                    xt[:],
                    idx_t[:, ib + k * szw : ib + (k + 1) * szw],
                    num_idxs=sz,
                    num_idxs_reg=sz,
                    elem_size=C,
                    transpose=True,
                    # single_packet=True corrupts data for >512-descriptor
                    # gathers (exceeds the 16KB SWDGE ring) — multi-packet
                    # mode is required for correctness at this size.
                    single_packet=False,
                )
                g[k] = gk

            pt = cpool.tile([P, TILE], mybir.dt.bfloat16, tag="p")
            nc.vector.tensor_tensor(
                out=pt[:, :sz], in0=g[0][:, :sz], in1=g[2][:, :sz], op=mybir.AluOpType.add
            )
            d13 = cpool.tile([P, TILE], mybir.dt.bfloat16, tag="d13")
            nc.vector.tensor_tensor(
                out=d13[:, :sz], in0=g[0][:, :sz], in1=g[2][:, :sz], op=mybir.AluOpType.subtract
            )
            qt = cpool.tile([P, TILE], mybir.dt.bfloat16, tag="q")
            nc.vector.tensor_tensor(
                out=qt[:, :sz], in0=g[1][:, :sz], in1=g[3][:, :sz], op=mybir.AluOpType.add
            )
            d24 = cpool.tile([P, TILE], mybir.dt.bfloat16, tag="d24")
            nc.vector.tensor_tensor(
                out=d24[:, :sz], in0=g[1][:, :sz], in1=g[3][:, :sz], op=mybir.AluOpType.subtract
            )
            a13 = cpool.tile([P, TILE], mybir.dt.bfloat16, tag="a13")
            nc.scalar.activation(
                out=a13[:, :sz], in_=d13[:, :sz], func=mybir.ActivationFunctionType.Abs
            )
            a24 = cpool.tile([P, TILE], mybir.dt.bfloat16, tag="a24")
            nc.scalar.activation(
                out=a24[:, :sz], in_=d24[:, :sz], func=mybir.ActivationFunctionType.Abs
            )

            taps = [(0, x0_t), (1, pt), (2, qt), (3, a13), (4, a24)]
            nch = (sz + CH - 1) // CH
            for ci in range(nch):
                w = min(CH, sz - ci * CH)
                col0 = toff[t] + ci * CH
                ncols = min(w, EH - col0)
                if ncols <= 0:
                    break
                for h in range(2):
                    ps = psum.tile([P, CH], mybir.dt.float32, tag=f"ps{h}")
                    for j, (k, rt) in enumerate(taps):
                        nc.tensor.matmul(
                            out=ps[:, :w],
                            lhsT=wt_t[:, k * CO + h * P : k * CO + h * P + P],
                            rhs=rt[:, ci * CH : ci * CH + w],
                            start=(j == 0),
                            stop=(j == len(taps) - 1),
                        )
                    ot = opool.tile([P, CH], mybir.dt.float32, tag=f"o{h}")
                    nc.scalar.activation(
                        out=ot[:, :w],
                        in_=ps[:, :w],
                        func=mybir.ActivationFunctionType.Identity,
                        bias=bias_t[:, h : h + 1],
                    )
                    nc.sync.dma_start(
                        out=out[h * P : (h + 1) * P, col0 : col0 + ncols],
                        in_=ot[:, :ncols],
                    )
    nc.finalize()
    return nc


def make_in_maps(x, ne_idx, conv_w, conv_b):
    xs = np.asarray(x)[..., 0]  # [B, C, E] f32
    xtb = np.ascontiguousarray(xs.transpose(0, 2, 1)).astype(BF16)  # [B, E, C]
    x0b = xs.astype(BF16)  # [B, C, E]

    wt_host = np.zeros((P, KT * CO), np.float32)
    for k in range(KT):
        wt_host[:, k * CO : (k + 1) * CO] = conv_w[:, :, 0, k].T
    wt_host = wt_host.astype(BF16)
    bias_host = np.ascontiguousarray(np.asarray(conv_b).reshape(2, P).T).astype(
        np.float32
    )

    in_maps = []
    for core in range(NCORES):
        b, h = divmod(core, 2)
        lo = h * EH
        x0c = np.zeros((C, EPAD), BF16)
        x0c[:, :EH] = x0b[b][:, lo : lo + EH]
        idxc = np.zeros((EPAD, 4), np.int16)
        idxc[:EH] = np.asarray(ne_idx)[b, lo : lo + EH, :].astype(np.int16)
        rep = np.zeros((NT, P, 4 * IDXW), np.int16)
        off = 0
        for t, sz in enumerate(TSZ):
            szw = sz // 16
            blk = idxc[off : off + sz].reshape(szw, 16, 4).transpose(1, 2, 0)
            rep[t, :, : 4 * szw] = np.broadcast_to(
                blk[None], (8, 16, 4, szw)
            ).reshape(P, 4 * szw)
            off += sz
        in_maps.append(
            {
                "xt": xtb[b],
                "x0": x0c,
                "idx": np.ascontiguousarray(
                    rep.transpose(1, 0, 2).reshape(P, NT * 4 * IDXW)
                ),
                "wt": wt_host,
                "bias": bias_host,
            }
        )
    return in_maps


def kernel(x, ne_idx, conv_w, conv_b):
    global _LAST_RESULTS, _PROGRAM
    from concourse.bass_utils import run_bass_kernel_spmd

    in_maps = make_in_maps(x, ne_idx, conv_w, conv_b)
    if _PROGRAM is None:
        _PROGRAM = build_program()
    res = run_bass_kernel_spmd(
        _PROGRAM,
        in_maps,
        core_ids=list(range(NCORES)),
        trace=bool(os.environ.get("KERNEL_TRACE")),
    )
    _LAST_RESULTS = res

    out_full = np.zeros((B, CO, E), np.float32)
    for core in range(NCORES):
        b, h = divmod(core, 2)
        out_full[b, :, h * EH : (h + 1) * EH] = res.results[core]["out"]
    return out_full[..., None]

